# revision 33
# baseline (speedup 1.0000x reference)
"""Trainium2 Bass kernel for additive-attention (nn_Attention_5789615915550).

reference math (per batch b):
    proj_f = features @ W1 + b1            # [L, U]
    proj_h = hidden[b] @ W2 + b2           # [U]
    score  = tanh(proj_f + proj_h)         # [L, U]
    logits = score @ Wv (+ bv)             # [L, 1]
    attn   = softmax(logits, axis=L)       # [L, 1]   (bv cancels in softmax)
    ctx    = sum_l attn[l] * features[l]   # [D]

Sharding: pure data parallel, batch dim B=1024 split across 8 cores (128 each).
All params replicated. Outputs gathered/reassembled on host.

Per-core dataflow (all matmul operands bf16, fp32 accumulation in PSUM):
  - features are cast-DMA'd (fp32->bf16) in groups of G=16 batches,
    natural layout [l_part, b, l_chunk, d].
  - PE transpose gives fT [d_part, d_chunk, l] per batch (rhs of W1 matmul).
  - proj psum [u_sub, u_chunk, l]; tanh on ACT with per-partition bias
    projh[u, b] (+b1+b2) fused via the activation bias operand.
  - logits: lhsT is a masked replica of Wv (only column bi nonzero), so all
    G batches accumulate their logit row into one [G, L] psum tile -- no
    partition-shifting copies needed.
  - softmax over L for G rows at once (exp's accum_out gives the sum free).
  - context: per batch 4 small matmuls, f-block as stationary weights,
    transposed attention column as moving operand; result collected as
    ctxT [D, B_c] and transposed on host.
"""

import os
import sys

import numpy as np

for _p in ("/opt/trn_rl_repo", "/root/.axon_site/_ro/trn_rl_repo"):
    if os.path.isdir(_p) and _p not in sys.path:
        sys.path.append(_p)

import concourse.bacc as bacc
import concourse.bass as bass
import concourse.tile as tile
from concourse import mybir
from concourse.bass_utils import run_bass_kernel_spmd
from concourse.masks import make_identity

B, L, D, H, U = 1024, 256, 256, 512, 512
NCORES = 8
BC = B // NCORES          # batches per core
G = 16                    # batches per softmax group
NG = BC // G              # groups per core
DC, UC, HC, LC = D // 128, U // 128, H // 128, L // 128  # 2, 4, 4, 2

F32 = mybir.dt.float32
BF16 = mybir.dt.bfloat16
AF = mybir.ActivationFunctionType


def _build_body(ctx, nc, tc, feats, hid, w1, b1, w2, b2, wv, ctx_out, attn_out):
    consts = ctx.enter_context(tc.tile_pool(name="consts", bufs=1))
    fpool = ctx.enter_context(tc.tile_pool(name="fpool", bufs=3))
    work = ctx.enter_context(tc.tile_pool(name="work", bufs=4))
    spool = ctx.enter_context(tc.tile_pool(name="spool", bufs=3))
    ps_tp = ctx.enter_context(tc.tile_pool(name="ps_tp", bufs=2, space="PSUM"))
    # proj psum in 1-bank tiles (2 u-chunks each) -> finer-grained release,
    # so tanh of one half unblocks the next batch's matmuls earlier.
    ps_proj = ctx.enter_context(tc.tile_pool(name="ps_proj", bufs=4, space="PSUM"))
    # logits accumulator and context rows have back-to-back lifetimes; share
    # one pool/tag so both fit in 2 banks.
    ps_log = ctx.enter_context(tc.tile_pool(name="ps_log", bufs=1, space="PSUM"))
    ps_ctx = ctx.enter_context(tc.tile_pool(name="ps_ctx", bufs=1, space="PSUM"))

    # ---- constants / setup -------------------------------------------------
    ident_bf = consts.tile([128, 128], BF16)
    make_identity(nc, ident_bf)

    # Weights arrive fp32 on the (otherwise idle) HWDGE ring, so the gpsimd
    # SWDGE ring can start streaming features immediately; DVE downcasts.
    w1_f = consts.tile([128, DC, U], F32)
    nc.sync.dma_start(out=w1_f, in_=w1.ap().rearrange("(c p) u -> p c u", p=128))
    w2_f = consts.tile([128, HC, U], F32)
    nc.sync.dma_start(out=w2_f, in_=w2.ap().rearrange("(c p) u -> p c u", p=128))
    hid_f = consts.tile([128, H], F32)  # [b, h]
    nc.sync.dma_start(out=hid_f, in_=hid.ap())
    w1_sb = consts.tile([128, DC, U], BF16)  # [d_sub, d_chunk, u]
    nc.vector.tensor_copy(w1_sb, w1_f)
    w2_sb = consts.tile([128, HC, U], BF16)  # [h_sub, h_chunk, u]
    nc.vector.tensor_copy(w2_sb, w2_f)
    hid_sb = consts.tile([128, H], BF16)  # [b, h]
    nc.vector.tensor_copy(hid_sb, hid_f)

    wv_f32 = consts.tile([128, UC, 1], F32)  # [u_sub, u_chunk, 1]
    nc.sync.dma_start(out=wv_f32, in_=wv.ap().rearrange("(c p) o -> p c o", p=128))
    b1_sb = consts.tile([128, UC], F32)
    nc.sync.dma_start(out=b1_sb, in_=b1.ap().rearrange("(c p) -> p c", p=128))
    b2_sb = consts.tile([128, UC], F32)
    nc.sync.dma_start(out=b2_sb, in_=b2.ap().rearrange("(c p) -> p c", p=128))
    bias12 = consts.tile([128, UC], F32)
    nc.vector.tensor_add(bias12, b1_sb, b2_sb)

    # Wv masked replicas: wv_mask[p, bi, c, j] = Wv[c*128+p] * (j == bi)
    wv_mask = consts.tile([128, G, UC, G], BF16)
    nc.vector.memset(wv_mask, 0.0)
    for bi in range(G):
        nc.vector.tensor_copy(wv_mask[:, bi, :, bi : bi + 1], wv_f32)

    # hidden -> hT (bf16), then projh[u, b] = W2.T @ hidden.T  (+ b1 + b2)
    hT_ps = ps_tp.tile([128, HC, 128], BF16, tag="tp")
    for hc in range(HC):
        nc.tensor.transpose(
            hT_ps[:, hc, :], hid_sb[:, hc * 128 : (hc + 1) * 128], ident_bf
        )
    hT_sb = consts.tile([128, HC, 128], BF16)  # [h_sub, h_chunk, b]
    nc.vector.tensor_copy(hT_sb, hT_ps)

    projh_sb = consts.tile([128, UC, BC], F32)  # [u_sub, u_chunk, b]
    for half in range(2):
        projh_ps = ps_proj.tile([128, 2, 128], F32, tag="proj")
        for u2 in range(2):
            uc = half * 2 + u2
            for hc in range(HC):
                nc.tensor.matmul(
                    projh_ps[:, u2, :],
                    lhsT=w2_sb[:, hc, uc * 128 : (uc + 1) * 128],
                    rhs=hT_sb[:, hc, :],
                    start=(hc == 0),
                    stop=(hc == HC - 1),
                )
        for u2 in range(2):
            uc = half * 2 + u2
            nc.vector.tensor_scalar_add(
                projh_sb[:, uc, :], projh_ps[:, u2, :], bias12[:, uc : uc + 1]
            )

    # Masked transposed-attention weights: atT_m[p, lc, bi, j] is nonzero only
    # at j == bi, holding attnT for batch b0+bi. Zeroed once; only the diagonal
    # columns are ever rewritten, so the zeros persist across groups.
    atT_m = consts.tile([128, LC, G, G], BF16)
    nc.vector.memset(atT_m, 0.0)

    # ---- main loop (software-pipelined: group g dense work, then group g-1
    # softmax/context which overlaps with it) ------------------------------
    fgrps = {}
    logaccs = {}

    def load_group(g, nchunks=2):
        fgrp = fpool.tile([128, G, LC, D], BF16, tag="fgrp")  # [l_sub, b, lc, d]
        fgrps[g] = fgrp
        b0 = g * G
        step = G // nchunks
        for h in range(nchunks):
            nc.gpsimd.dma_start(
                out=fgrp[:, h * step : (h + 1) * step],
                in_=feats.ap()[b0 + h * step : b0 + (h + 1) * step].rearrange(
                    "b (c p) d -> p b c d", p=128
                ),
            )

    def dense_stage(g):
        b0 = g * G
        if g + 1 < NG:
            load_group(g + 1)
        fgrp = fgrps[g]
        logacc_ps = ps_log.tile([G, L], F32, tag="log")
        logaccs[g] = logacc_ps

        def emit_logits(bi, score):
            for uc in range(UC):
                nc.tensor.matmul(
                    logacc_ps,
                    lhsT=wv_mask[:, bi, uc, :],
                    rhs=score[:, uc, :],
                    start=(bi == 0 and uc == 0),
                    stop=(bi == G - 1 and uc == UC - 1),
                )

        pending = None  # (bi, score) whose logits are deferred one batch
        for bi in range(G):
            b = b0 + bi
            # transpose f[b]: [l, d] -> fT [d_sub, d_chunk, l]
            ft_ps = ps_tp.tile([128, DC, L], BF16, tag="tp")
            for lc in range(LC):
                for dc in range(DC):
                    nc.tensor.transpose(
                        ft_ps[:, dc, lc * 128 : (lc + 1) * 128],
                        fgrp[:, bi, lc, dc * 128 : (dc + 1) * 128],
                        ident_bf,
                    )
            ft_sb = work.tile([128, DC, L], BF16, tag="ft")
            nc.vector.tensor_copy(ft_sb, ft_ps)

            # proj_f[u, l] for this batch, two psum tiles of 2 u-chunks each
            score = work.tile([128, UC, L], BF16, tag="score")
            for half in range(2):
                proj_ps = ps_proj.tile([128, 2, L], F32, tag="proj")
                for u2 in range(2):
                    uc = half * 2 + u2
                    for dc in range(DC):
                        nc.tensor.matmul(
                            proj_ps[:, u2, :],
                            lhsT=w1_sb[:, dc, uc * 128 : (uc + 1) * 128],
                            rhs=ft_sb[:, dc, :],
                            start=(dc == 0),
                            stop=(dc == DC - 1),
                        )
                # score = tanh(proj_f + projh[:, b] + b1 + b2)
                for u2 in range(2):
                    uc = half * 2 + u2
                    nc.scalar.activation(
                        out=score[:, uc, :],
                        in_=proj_ps[:, u2, :],
                        func=AF.Tanh,
                        bias=projh_sb[:, uc, b : b + 1],
                        scale=1.0,
                    )

            # logits lag one batch so tanh(b) overlaps proj(b+1) on the PE
            if pending is not None:
                emit_logits(*pending)
            pending = (bi, score)
        emit_logits(*pending)

    def reduce_stage(g):
        b0 = g * G
        logacc_ps = logaccs.pop(g)
        fgrp = fgrps.pop(g)
        # ---- batched softmax over L for the G rows (read psum directly) ----
        mx = spool.tile([G, 1], F32, tag="mx")
        nc.vector.reduce_max(mx, logacc_ps, axis=mybir.AxisListType.X)
        nmx = spool.tile([G, 1], F32, tag="nmx")
        nc.vector.tensor_scalar_mul(nmx, mx, -1.0)
        probs = spool.tile([G, L], F32, tag="probs")
        ssum = spool.tile([G, 1], F32, tag="ssum")
        nc.scalar.activation(
            out=probs,
            in_=logacc_ps,
            func=AF.Exp,
            bias=nmx,
            scale=1.0,
            accum_out=ssum,
        )
        rec = spool.tile([G, 1], F32, tag="rec")
        nc.vector.reciprocal(rec, ssum)
        attn_f = spool.tile([G, L], F32, tag="attnf")
        nc.vector.tensor_scalar_mul(attn_f, probs, rec)
        nc.sync.dma_start(out=attn_out.ap()[b0 : b0 + G, :], in_=attn_f)
        attn_b = spool.tile([G, L], BF16, tag="attnb")
        nc.vector.tensor_scalar_mul(attn_b, probs, rec)

        # attn -> attnT [l_sub, l_chunk, b_in_group], scattered to diagonal
        at_ps = ps_tp.tile([128, LC, G], BF16, tag="tp")
        for lc in range(LC):
            nc.tensor.transpose(
                at_ps[:, lc, :],
                attn_b[:, lc * 128 : (lc + 1) * 128],
                ident_bf[0:G, 0:G],
            )
        for bi in range(G):
            nc.vector.tensor_copy(atT_m[:, :, bi, bi], at_ps[:, :, bi])

        # context rows: ctx[bi, d] = sum_l attn[b0+bi, l] f[b0+bi, l, d]
        ctx_ps = ps_ctx.tile([G, D], F32, tag="ctx")
        for bi in range(G):
            for lc in range(LC):
                nc.tensor.matmul(
                    ctx_ps,
                    lhsT=atT_m[:, lc, bi, :],
                    rhs=fgrp[:, bi, lc, :],
                    start=(bi == 0 and lc == 0),
                    stop=(bi == G - 1 and lc == LC - 1),
                )
        ctx_sb = spool.tile([G, D], F32, tag="ctxsb")
        nc.vector.tensor_copy(ctx_sb, ctx_ps)
        nc.sync.dma_start(out=ctx_out.ap()[b0 : b0 + G, :], in_=ctx_sb)

    load_group(0, nchunks=4)
    for g in range(NG):
        dense_stage(g)
        if g > 0:
            reduce_stage(g - 1)
    reduce_stage(NG - 1)


_NC_CACHE = {}


def _get_nc():
    if "nc" not in _NC_CACHE:
        nc = bacc.Bacc("TRN2", target_bir_lowering=False, debug=False)
        feats = nc.declare_dram_parameter("features", [BC, L, D], F32, isOutput=False)
        hid = nc.declare_dram_parameter("hidden", [BC, H], F32, isOutput=False)
        w1 = nc.declare_dram_parameter("W1", [D, U], F32, isOutput=False)
        b1 = nc.declare_dram_parameter("b1", [U], F32, isOutput=False)
        w2 = nc.declare_dram_parameter("W2", [H, U], F32, isOutput=False)
        b2 = nc.declare_dram_parameter("b2", [U], F32, isOutput=False)
        wv = nc.declare_dram_parameter("Wv", [U, 1], F32, isOutput=False)
        ctx_out = nc.declare_dram_parameter("ctx", [BC, D], F32, isOutput=True)
        attn_out = nc.declare_dram_parameter("attn", [BC, L], F32, isOutput=True)
        from contextlib import ExitStack

        with tile.TileContext(nc) as tc, ExitStack() as es:
            _build_body(es, nc, tc, feats, hid, w1, b1, w2, b2, wv, ctx_out, attn_out)
        nc.compile()
        _NC_CACHE["nc"] = nc
    return _NC_CACHE["nc"]


def run(inputs, **spmd_kwargs):
    """Shard inputs, run on 8 NeuronCores, gather. Returns (results_obj, outputs)."""
    nc = _get_nc()
    f = np.ascontiguousarray(np.asarray(inputs["features"], dtype=np.float32))
    h = np.ascontiguousarray(np.asarray(inputs["hidden"], dtype=np.float32))
    shared = {
        "W1": np.ascontiguousarray(np.asarray(inputs["W1"], dtype=np.float32)),
        "b1": np.ascontiguousarray(np.asarray(inputs["b1"], dtype=np.float32)),
        "W2": np.ascontiguousarray(np.asarray(inputs["W2"], dtype=np.float32)),
        "b2": np.ascontiguousarray(np.asarray(inputs["b2"], dtype=np.float32)),
        "Wv": np.ascontiguousarray(np.asarray(inputs["Wv"], dtype=np.float32)),
    }
    in_maps = [
        {
            "features": f[i * BC : (i + 1) * BC],
            "hidden": h[i * BC : (i + 1) * BC],
            **shared,
        }
        for i in range(NCORES)
    ]
    res = run_bass_kernel_spmd(nc, in_maps, core_ids=list(range(NCORES)), **spmd_kwargs)
    ctx = np.concatenate([r["ctx"] for r in res.results], axis=0).astype(np.float32)
    attn = np.concatenate([r["attn"] for r in res.results], axis=0)[..., None].astype(
        np.float32
    )
    return res, (ctx, attn)


def kernel(features, hidden, W1, b1, W2, b2, Wv, bv):
    del bv  # softmax(logits + bv) == softmax(logits); bv never affects outputs
    _, out = run(
        {
            "features": features,
            "hidden": hidden,
            "W1": W1,
            "b1": b1,
            "W2": W2,
            "b2": b2,
            "Wv": Wv,
        }
    )
    return out


# revision 34
# speedup vs baseline: 1.0206x; 1.0206x over previous
"""Trainium2 Bass kernel for additive-attention (nn_Attention_5789615915550).

reference math (per batch b):
    proj_f = features @ W1 + b1            # [L, U]
    proj_h = hidden[b] @ W2 + b2           # [U]
    score  = tanh(proj_f + proj_h)         # [L, U]
    logits = score @ Wv (+ bv)             # [L, 1]
    attn   = softmax(logits, axis=L)       # [L, 1]   (bv cancels in softmax)
    ctx    = sum_l attn[l] * features[l]   # [D]

Sharding: pure data parallel, batch dim B=1024 split across 8 cores (128 each).
All params replicated. Outputs gathered/reassembled on host.

Per-core dataflow (all matmul operands bf16, fp32 accumulation in PSUM):
  - features are cast-DMA'd (fp32->bf16) in groups of G=16 batches,
    natural layout [l_part, b, l_chunk, d].
  - PE transpose gives fT [d_part, d_chunk, l] per batch (rhs of W1 matmul).
  - proj psum [u_sub, u_chunk, l]; tanh on ACT with per-partition bias
    projh[u, b] (+b1+b2) fused via the activation bias operand.
  - logits: lhsT is a masked replica of Wv (only column bi nonzero), so all
    G batches accumulate their logit row into one [G, L] psum tile -- no
    partition-shifting copies needed.
  - softmax over L for G rows at once (exp's accum_out gives the sum free).
  - context: per batch 4 small matmuls, f-block as stationary weights,
    transposed attention column as moving operand; result collected as
    ctxT [D, B_c] and transposed on host.
"""

import os
import sys

import numpy as np

for _p in ("/opt/trn_rl_repo", "/root/.axon_site/_ro/trn_rl_repo"):
    if os.path.isdir(_p) and _p not in sys.path:
        sys.path.append(_p)

import concourse.bacc as bacc
import concourse.bass as bass
import concourse.tile as tile
from concourse import mybir
from concourse.bass_utils import run_bass_kernel_spmd
from concourse.masks import make_identity

B, L, D, H, U = 1024, 256, 256, 512, 512
NCORES = 8
BC = B // NCORES          # batches per core
G = 16                    # batches per softmax group
NG = BC // G              # groups per core
DC, UC, HC, LC = D // 128, U // 128, H // 128, L // 128  # 2, 4, 4, 2

F32 = mybir.dt.float32
BF16 = mybir.dt.bfloat16
AF = mybir.ActivationFunctionType


def _build_body(ctx, nc, tc, feats, hid, w1, b1, w2, b2, wv, ctx_out, attn_out):
    consts = ctx.enter_context(tc.tile_pool(name="consts", bufs=1))
    fpool = ctx.enter_context(tc.tile_pool(name="fpool", bufs=3))
    work = ctx.enter_context(tc.tile_pool(name="work", bufs=4))
    spool = ctx.enter_context(tc.tile_pool(name="spool", bufs=3))
    ps_tp = ctx.enter_context(tc.tile_pool(name="ps_tp", bufs=2, space="PSUM"))
    # proj psum in 1-bank tiles (2 u-chunks each) -> finer-grained release,
    # so tanh of one half unblocks the next batch's matmuls earlier.
    ps_proj = ctx.enter_context(tc.tile_pool(name="ps_proj", bufs=2, space="PSUM"))
    # logits accumulator and context rows have back-to-back lifetimes; share
    # one pool/tag so both fit in 2 banks.
    ps_log = ctx.enter_context(tc.tile_pool(name="ps_log", bufs=1, space="PSUM"))
    ps_ctx = ctx.enter_context(tc.tile_pool(name="ps_ctx", bufs=1, space="PSUM"))

    # ---- constants / setup -------------------------------------------------
    ident_bf = consts.tile([128, 128], BF16)
    make_identity(nc, ident_bf)

    # Weights arrive fp32 on the (otherwise idle) HWDGE ring, so the gpsimd
    # SWDGE ring can start streaming features immediately; DVE downcasts.
    w1_f = consts.tile([128, DC, U], F32)
    nc.sync.dma_start(out=w1_f, in_=w1.ap().rearrange("(c p) u -> p c u", p=128))
    w2_f = consts.tile([128, HC, U], F32)
    nc.sync.dma_start(out=w2_f, in_=w2.ap().rearrange("(c p) u -> p c u", p=128))
    hid_f = consts.tile([128, H], F32)  # [b, h]
    nc.sync.dma_start(out=hid_f, in_=hid.ap())
    w1_sb = consts.tile([128, DC, U], BF16)  # [d_sub, d_chunk, u]
    nc.vector.tensor_copy(w1_sb, w1_f)
    w2_sb = consts.tile([128, HC, U], BF16)  # [h_sub, h_chunk, u]
    nc.vector.tensor_copy(w2_sb, w2_f)
    hid_sb = consts.tile([128, H], BF16)  # [b, h]
    nc.vector.tensor_copy(hid_sb, hid_f)

    wv_f32 = consts.tile([128, UC, 1], F32)  # [u_sub, u_chunk, 1]
    nc.sync.dma_start(out=wv_f32, in_=wv.ap().rearrange("(c p) o -> p c o", p=128))
    b1_sb = consts.tile([128, UC], F32)
    nc.sync.dma_start(out=b1_sb, in_=b1.ap().rearrange("(c p) -> p c", p=128))
    b2_sb = consts.tile([128, UC], F32)
    nc.sync.dma_start(out=b2_sb, in_=b2.ap().rearrange("(c p) -> p c", p=128))
    bias12 = consts.tile([128, UC], F32)
    nc.vector.tensor_add(bias12, b1_sb, b2_sb)

    # Wv masked replicas: wv_mask[p, bi, c, j] = Wv[c*128+p] * (j == bi)
    wv_mask = consts.tile([128, G, UC, G], BF16)
    nc.vector.memset(wv_mask, 0.0)
    for bi in range(G):
        nc.vector.tensor_copy(wv_mask[:, bi, :, bi : bi + 1], wv_f32)

    # hidden -> hT (bf16), then projh[u, b] = W2.T @ hidden.T  (+ b1 + b2)
    hT_ps = ps_tp.tile([128, HC, 128], BF16, tag="tp")
    for hc in range(HC):
        nc.tensor.transpose(
            hT_ps[:, hc, :], hid_sb[:, hc * 128 : (hc + 1) * 128], ident_bf
        )
    hT_sb = consts.tile([128, HC, 128], BF16)  # [h_sub, h_chunk, b]
    nc.vector.tensor_copy(hT_sb, hT_ps)

    projh_sb = consts.tile([128, UC, BC], F32)  # [u_sub, u_chunk, b]
    projh_ps = ps_proj.tile([128, UC, 128], F32, tag="proj")
    for uc in range(UC):
        for hc in range(HC):
            nc.tensor.matmul(
                projh_ps[:, uc, :],
                lhsT=w2_sb[:, hc, uc * 128 : (uc + 1) * 128],
                rhs=hT_sb[:, hc, :],
                start=(hc == 0),
                stop=(hc == HC - 1),
            )
    for uc in range(UC):
        nc.vector.tensor_scalar_add(
            projh_sb[:, uc, :], projh_ps[:, uc, :], bias12[:, uc : uc + 1]
        )

    # Masked transposed-attention weights: atT_m[p, lc, bi, j] is nonzero only
    # at j == bi, holding attnT for batch b0+bi. Zeroed once; only the diagonal
    # columns are ever rewritten, so the zeros persist across groups.
    atT_m = consts.tile([128, LC, G, G], BF16)
    nc.vector.memset(atT_m, 0.0)

    # ---- main loop (software-pipelined: group g dense work, then group g-1
    # softmax/context which overlaps with it) ------------------------------
    fgrps = {}
    logaccs = {}

    def load_group(g, nchunks=2):
        fgrp = fpool.tile([128, G, LC, D], BF16, tag="fgrp")  # [l_sub, b, lc, d]
        fgrps[g] = fgrp
        b0 = g * G
        step = G // nchunks
        for h in range(nchunks):
            nc.gpsimd.dma_start(
                out=fgrp[:, h * step : (h + 1) * step],
                in_=feats.ap()[b0 + h * step : b0 + (h + 1) * step].rearrange(
                    "b (c p) d -> p b c d", p=128
                ),
            )

    def dense_stage(g):
        b0 = g * G
        if g + 1 < NG:
            load_group(g + 1)
        fgrp = fgrps[g]
        logacc_ps = ps_log.tile([G, L], F32, tag="log")
        logaccs[g] = logacc_ps

        def emit_logits(bi, score):
            for uc in range(UC):
                nc.tensor.matmul(
                    logacc_ps,
                    lhsT=wv_mask[:, bi, uc, :],
                    rhs=score[:, uc, :],
                    start=(bi == 0 and uc == 0),
                    stop=(bi == G - 1 and uc == UC - 1),
                )

        pending = None  # (bi, score) whose logits are deferred one batch
        for bi in range(G):
            b = b0 + bi
            # transpose f[b]: [l, d] -> fT [d_sub, d_chunk, l]
            ft_ps = ps_tp.tile([128, DC, L], BF16, tag="tp")
            for lc in range(LC):
                for dc in range(DC):
                    nc.tensor.transpose(
                        ft_ps[:, dc, lc * 128 : (lc + 1) * 128],
                        fgrp[:, bi, lc, dc * 128 : (dc + 1) * 128],
                        ident_bf,
                    )
            ft_sb = work.tile([128, DC, L], BF16, tag="ft")
            nc.vector.tensor_copy(ft_sb, ft_ps)

            # proj_f[u, l] for this batch
            proj_ps = ps_proj.tile([128, UC, L], F32, tag="proj")
            for uc in range(UC):
                for dc in range(DC):
                    nc.tensor.matmul(
                        proj_ps[:, uc, :],
                        lhsT=w1_sb[:, dc, uc * 128 : (uc + 1) * 128],
                        rhs=ft_sb[:, dc, :],
                        start=(dc == 0),
                        stop=(dc == DC - 1),
                    )
            # score = tanh(proj_f + projh[:, b] + b1 + b2)
            score = work.tile([128, UC, L], BF16, tag="score")
            for uc in range(UC):
                nc.scalar.activation(
                    out=score[:, uc, :],
                    in_=proj_ps[:, uc, :],
                    func=AF.Tanh,
                    bias=projh_sb[:, uc, b : b + 1],
                    scale=1.0,
                )

            # logits lag one batch so tanh(b) overlaps proj(b+1) on the PE
            if pending is not None:
                emit_logits(*pending)
            pending = (bi, score)
        emit_logits(*pending)

    def reduce_stage(g):
        b0 = g * G
        logacc_ps = logaccs.pop(g)
        fgrp = fgrps.pop(g)
        # ---- batched softmax over L for the G rows (read psum directly) ----
        mx = spool.tile([G, 1], F32, tag="mx")
        nc.vector.reduce_max(mx, logacc_ps, axis=mybir.AxisListType.X)
        nmx = spool.tile([G, 1], F32, tag="nmx")
        nc.vector.tensor_scalar_mul(nmx, mx, -1.0)
        probs = spool.tile([G, L], F32, tag="probs")
        ssum = spool.tile([G, 1], F32, tag="ssum")
        nc.scalar.activation(
            out=probs,
            in_=logacc_ps,
            func=AF.Exp,
            bias=nmx,
            scale=1.0,
            accum_out=ssum,
        )
        rec = spool.tile([G, 1], F32, tag="rec")
        nc.vector.reciprocal(rec, ssum)
        attn_f = spool.tile([G, L], F32, tag="attnf")
        nc.vector.tensor_scalar_mul(attn_f, probs, rec)
        nc.sync.dma_start(out=attn_out.ap()[b0 : b0 + G, :], in_=attn_f)
        attn_b = spool.tile([G, L], BF16, tag="attnb")
        nc.vector.tensor_scalar_mul(attn_b, probs, rec)

        # attn -> attnT [l_sub, l_chunk, b_in_group], scattered to diagonal
        at_ps = ps_tp.tile([128, LC, G], BF16, tag="tp")
        for lc in range(LC):
            nc.tensor.transpose(
                at_ps[:, lc, :],
                attn_b[:, lc * 128 : (lc + 1) * 128],
                ident_bf[0:G, 0:G],
            )
        for bi in range(G):
            nc.vector.tensor_copy(atT_m[:, :, bi, bi], at_ps[:, :, bi])

        # context rows: ctx[bi, d] = sum_l attn[b0+bi, l] f[b0+bi, l, d]
        ctx_ps = ps_ctx.tile([G, D], F32, tag="ctx")
        for bi in range(G):
            for lc in range(LC):
                nc.tensor.matmul(
                    ctx_ps,
                    lhsT=atT_m[:, lc, bi, :],
                    rhs=fgrp[:, bi, lc, :],
                    start=(bi == 0 and lc == 0),
                    stop=(bi == G - 1 and lc == LC - 1),
                )
        ctx_sb = spool.tile([G, D], F32, tag="ctxsb")
        nc.vector.tensor_copy(ctx_sb, ctx_ps)
        nc.sync.dma_start(out=ctx_out.ap()[b0 : b0 + G, :], in_=ctx_sb)

    load_group(0, nchunks=4)
    for g in range(NG):
        dense_stage(g)
        if g > 0:
            reduce_stage(g - 1)
    reduce_stage(NG - 1)


_NC_CACHE = {}


def _get_nc():
    if "nc" not in _NC_CACHE:
        nc = bacc.Bacc("TRN2", target_bir_lowering=False, debug=False)
        feats = nc.declare_dram_parameter("features", [BC, L, D], F32, isOutput=False)
        hid = nc.declare_dram_parameter("hidden", [BC, H], F32, isOutput=False)
        w1 = nc.declare_dram_parameter("W1", [D, U], F32, isOutput=False)
        b1 = nc.declare_dram_parameter("b1", [U], F32, isOutput=False)
        w2 = nc.declare_dram_parameter("W2", [H, U], F32, isOutput=False)
        b2 = nc.declare_dram_parameter("b2", [U], F32, isOutput=False)
        wv = nc.declare_dram_parameter("Wv", [U, 1], F32, isOutput=False)
        ctx_out = nc.declare_dram_parameter("ctx", [BC, D], F32, isOutput=True)
        attn_out = nc.declare_dram_parameter("attn", [BC, L], F32, isOutput=True)
        from contextlib import ExitStack

        with tile.TileContext(nc) as tc, ExitStack() as es:
            _build_body(es, nc, tc, feats, hid, w1, b1, w2, b2, wv, ctx_out, attn_out)
        nc.compile()
        _NC_CACHE["nc"] = nc
    return _NC_CACHE["nc"]


def run(inputs, **spmd_kwargs):
    """Shard inputs, run on 8 NeuronCores, gather. Returns (results_obj, outputs)."""
    nc = _get_nc()
    f = np.ascontiguousarray(np.asarray(inputs["features"], dtype=np.float32))
    h = np.ascontiguousarray(np.asarray(inputs["hidden"], dtype=np.float32))
    shared = {
        "W1": np.ascontiguousarray(np.asarray(inputs["W1"], dtype=np.float32)),
        "b1": np.ascontiguousarray(np.asarray(inputs["b1"], dtype=np.float32)),
        "W2": np.ascontiguousarray(np.asarray(inputs["W2"], dtype=np.float32)),
        "b2": np.ascontiguousarray(np.asarray(inputs["b2"], dtype=np.float32)),
        "Wv": np.ascontiguousarray(np.asarray(inputs["Wv"], dtype=np.float32)),
    }
    in_maps = [
        {
            "features": f[i * BC : (i + 1) * BC],
            "hidden": h[i * BC : (i + 1) * BC],
            **shared,
        }
        for i in range(NCORES)
    ]
    res = run_bass_kernel_spmd(nc, in_maps, core_ids=list(range(NCORES)), **spmd_kwargs)
    ctx = np.concatenate([r["ctx"] for r in res.results], axis=0).astype(np.float32)
    attn = np.concatenate([r["attn"] for r in res.results], axis=0)[..., None].astype(
        np.float32
    )
    return res, (ctx, attn)


def kernel(features, hidden, W1, b1, W2, b2, Wv, bv):
    del bv  # softmax(logits + bv) == softmax(logits); bv never affects outputs
    _, out = run(
        {
            "features": features,
            "hidden": hidden,
            "W1": W1,
            "b1": b1,
            "W2": W2,
            "b2": b2,
            "Wv": Wv,
        }
    )
    return out


# revision 35
# speedup vs baseline: 1.0582x; 1.0368x over previous
"""Trainium2 Bass kernel for additive-attention (nn_Attention_5789615915550).

reference math (per batch b):
    proj_f = features @ W1 + b1            # [L, U]
    proj_h = hidden[b] @ W2 + b2           # [U]
    score  = tanh(proj_f + proj_h)         # [L, U]
    logits = score @ Wv (+ bv)             # [L, 1]
    attn   = softmax(logits, axis=L)       # [L, 1]   (bv cancels in softmax)
    ctx    = sum_l attn[l] * features[l]   # [D]

Sharding: pure data parallel, batch dim B=1024 split across 8 cores (128 each).
All params replicated. Outputs gathered/reassembled on host.

Per-core dataflow (all matmul operands bf16, fp32 accumulation in PSUM):
  - features are cast-DMA'd (fp32->bf16) in groups of G=16 batches,
    natural layout [l_part, b, l_chunk, d].
  - PE transpose gives fT [d_part, d_chunk, l] per batch (rhs of W1 matmul).
  - proj psum [u_sub, u_chunk, l]; tanh on ACT with per-partition bias
    projh[u, b] (+b1+b2) fused via the activation bias operand.
  - logits: lhsT is a masked replica of Wv (only column bi nonzero), so all
    G batches accumulate their logit row into one [G, L] psum tile -- no
    partition-shifting copies needed.
  - softmax over L for G rows at once (exp's accum_out gives the sum free).
  - context: per batch 4 small matmuls, f-block as stationary weights,
    transposed attention column as moving operand; result collected as
    ctxT [D, B_c] and transposed on host.
"""

import os
import sys

import numpy as np

for _p in ("/opt/trn_rl_repo", "/root/.axon_site/_ro/trn_rl_repo"):
    if os.path.isdir(_p) and _p not in sys.path:
        sys.path.append(_p)

import concourse.bacc as bacc
import concourse.bass as bass
import concourse.tile as tile
from concourse import mybir
from concourse.bass_utils import run_bass_kernel_spmd
from concourse.masks import make_identity

B, L, D, H, U = 1024, 256, 256, 512, 512
NCORES = 8
BC = B // NCORES          # batches per core
G = 16                    # batches per softmax group
NG = BC // G              # groups per core
DC, UC, HC, LC = D // 128, U // 128, H // 128, L // 128  # 2, 4, 4, 2

F32 = mybir.dt.float32
BF16 = mybir.dt.bfloat16
AF = mybir.ActivationFunctionType


def _build_body(ctx, nc, tc, feats, hid, w1, b1, w2, b2, wv, ctx_out, attn_out):
    consts = ctx.enter_context(tc.tile_pool(name="consts", bufs=1))
    fpool = ctx.enter_context(tc.tile_pool(name="fpool", bufs=3))
    work = ctx.enter_context(tc.tile_pool(name="work", bufs=4))
    spool = ctx.enter_context(tc.tile_pool(name="spool", bufs=3))
    ps_tp = ctx.enter_context(tc.tile_pool(name="ps_tp", bufs=2, space="PSUM"))
    # proj psum in 1-bank tiles (2 u-chunks each) -> finer-grained release,
    # so tanh of one half unblocks the next batch's matmuls earlier.
    ps_proj = ctx.enter_context(tc.tile_pool(name="ps_proj", bufs=2, space="PSUM"))
    # logits accumulator and context rows have back-to-back lifetimes; share
    # one pool/tag so both fit in 2 banks.
    ps_log = ctx.enter_context(tc.tile_pool(name="ps_log", bufs=1, space="PSUM"))
    ps_ctx = ctx.enter_context(tc.tile_pool(name="ps_ctx", bufs=1, space="PSUM"))

    # ---- constants / setup -------------------------------------------------
    ident_bf = consts.tile([128, 128], BF16)
    make_identity(nc, ident_bf)

    w1_sb = consts.tile([128, DC, U], BF16)  # [d_sub, d_chunk, u]
    nc.gpsimd.dma_start(out=w1_sb, in_=w1.ap().rearrange("(c p) u -> p c u", p=128))
    w2_sb = consts.tile([128, HC, U], BF16)  # [h_sub, h_chunk, u]
    nc.gpsimd.dma_start(out=w2_sb, in_=w2.ap().rearrange("(c p) u -> p c u", p=128))

    wv_f32 = consts.tile([128, UC, 1], F32)  # [u_sub, u_chunk, 1]
    nc.sync.dma_start(out=wv_f32, in_=wv.ap().rearrange("(c p) o -> p c o", p=128))
    b1_sb = consts.tile([128, UC], F32)
    nc.sync.dma_start(out=b1_sb, in_=b1.ap().rearrange("(c p) -> p c", p=128))
    b2_sb = consts.tile([128, UC], F32)
    nc.sync.dma_start(out=b2_sb, in_=b2.ap().rearrange("(c p) -> p c", p=128))
    bias12 = consts.tile([128, UC], F32)
    nc.vector.tensor_add(bias12, b1_sb, b2_sb)

    # Wv masked replicas: wv_mask[p, bi, c, j] = Wv[c*128+p] * (j == bi)
    wv_mask = consts.tile([128, G, UC, G], BF16)
    nc.vector.memset(wv_mask, 0.0)
    for bi in range(G):
        nc.vector.tensor_copy(wv_mask[:, bi, :, bi : bi + 1], wv_f32)

    # hidden -> hT (bf16), then projh[u, b] = W2.T @ hidden.T  (+ b1 + b2)
    hid_sb = consts.tile([128, H], BF16)  # [b, h]
    nc.gpsimd.dma_start(out=hid_sb, in_=hid.ap())
    hT_ps = ps_tp.tile([128, HC, 128], BF16, tag="tp")
    for hc in range(HC):
        nc.tensor.transpose(
            hT_ps[:, hc, :], hid_sb[:, hc * 128 : (hc + 1) * 128], ident_bf
        )
    hT_sb = consts.tile([128, HC, 128], BF16)  # [h_sub, h_chunk, b]
    nc.vector.tensor_copy(hT_sb, hT_ps)

    projh_sb = consts.tile([128, UC, BC], F32)  # [u_sub, u_chunk, b]
    projh_ps = ps_proj.tile([128, UC, 128], F32, tag="proj")
    for uc in range(UC):
        for hc in range(HC):
            nc.tensor.matmul(
                projh_ps[:, uc, :],
                lhsT=w2_sb[:, hc, uc * 128 : (uc + 1) * 128],
                rhs=hT_sb[:, hc, :],
                start=(hc == 0),
                stop=(hc == HC - 1),
            )
    for uc in range(UC):
        nc.vector.tensor_scalar_add(
            projh_sb[:, uc, :], projh_ps[:, uc, :], bias12[:, uc : uc + 1]
        )

    # Masked transposed-attention weights: atT_m[p, lc, bi, j] is nonzero only
    # at j == bi, holding attnT for batch b0+bi. Zeroed once; only the diagonal
    # columns are ever rewritten, so the zeros persist across groups.
    atT_m = consts.tile([128, LC, G, G], BF16)
    nc.vector.memset(atT_m, 0.0)

    # ---- main loop (software-pipelined: group g dense work, then group g-1
    # softmax/context which overlaps with it) ------------------------------
    fgrps = {}
    logaccs = {}

    def load_group(g, nchunks=2):
        fgrp = fpool.tile([128, G, LC, D], BF16, tag="fgrp")  # [l_sub, b, lc, d]
        fgrps[g] = fgrp
        b0 = g * G
        step = G // nchunks
        for h in range(nchunks):
            nc.gpsimd.dma_start(
                out=fgrp[:, h * step : (h + 1) * step],
                in_=feats.ap()[b0 + h * step : b0 + (h + 1) * step].rearrange(
                    "b (c p) d -> p b c d", p=128
                ),
            )

    def dense_stage(g):
        b0 = g * G
        if g + 1 < NG:
            load_group(g + 1)
        fgrp = fgrps[g]
        logacc_ps = ps_log.tile([G, L], F32, tag="log")
        logaccs[g] = logacc_ps

        def emit_logits(bi, score):
            for uc in range(UC):
                nc.tensor.matmul(
                    logacc_ps,
                    lhsT=wv_mask[:, bi, uc, :],
                    rhs=score[:, uc, :],
                    start=(bi == 0 and uc == 0),
                    stop=(bi == G - 1 and uc == UC - 1),
                )

        pending = None  # (bi, score) whose logits are deferred one batch
        for bi in range(G):
            b = b0 + bi
            # transpose f[b]: [l, d] -> fT [d_sub, d_chunk, l]
            ft_ps = ps_tp.tile([128, DC, L], BF16, tag="tp")
            for lc in range(LC):
                for dc in range(DC):
                    nc.tensor.transpose(
                        ft_ps[:, dc, lc * 128 : (lc + 1) * 128],
                        fgrp[:, bi, lc, dc * 128 : (dc + 1) * 128],
                        ident_bf,
                    )
            ft_sb = work.tile([128, DC, L], BF16, tag="ft")
            nc.vector.tensor_copy(ft_sb, ft_ps)

            # proj_f[u, l] for this batch
            proj_ps = ps_proj.tile([128, UC, L], F32, tag="proj")
            for uc in range(UC):
                for dc in range(DC):
                    nc.tensor.matmul(
                        proj_ps[:, uc, :],
                        lhsT=w1_sb[:, dc, uc * 128 : (uc + 1) * 128],
                        rhs=ft_sb[:, dc, :],
                        start=(dc == 0),
                        stop=(dc == DC - 1),
                    )
            # score = tanh(proj_f + projh[:, b] + b1 + b2)
            score = work.tile([128, UC, L], BF16, tag="score")
            for uc in range(UC):
                nc.scalar.activation(
                    out=score[:, uc, :],
                    in_=proj_ps[:, uc, :],
                    func=AF.Tanh,
                    bias=projh_sb[:, uc, b : b + 1],
                    scale=1.0,
                )

            # logits lag one batch so tanh(b) overlaps proj(b+1) on the PE
            if pending is not None:
                emit_logits(*pending)
            pending = (bi, score)
        emit_logits(*pending)

    def reduce_stage(g):
        b0 = g * G
        logacc_ps = logaccs.pop(g)
        fgrp = fgrps.pop(g)
        # ---- batched softmax over L for the G rows (read psum directly) ----
        mx = spool.tile([G, 1], F32, tag="mx")
        nc.vector.reduce_max(mx, logacc_ps, axis=mybir.AxisListType.X)
        nmx = spool.tile([G, 1], F32, tag="nmx")
        nc.vector.tensor_scalar_mul(nmx, mx, -1.0)
        probs = spool.tile([G, L], F32, tag="probs")
        ssum = spool.tile([G, 1], F32, tag="ssum")
        nc.scalar.activation(
            out=probs,
            in_=logacc_ps,
            func=AF.Exp,
            bias=nmx,
            scale=1.0,
            accum_out=ssum,
        )
        rec = spool.tile([G, 1], F32, tag="rec")
        nc.vector.reciprocal(rec, ssum)
        attn_f = spool.tile([G, L], F32, tag="attnf")
        nc.vector.tensor_scalar_mul(attn_f, probs, rec)
        nc.sync.dma_start(out=attn_out.ap()[b0 : b0 + G, :], in_=attn_f)
        attn_b = spool.tile([G, L], BF16, tag="attnb")
        nc.vector.tensor_scalar_mul(attn_b, probs, rec)

        # attn -> attnT [l_sub, l_chunk, b_in_group], scattered to diagonal
        at_ps = ps_tp.tile([128, LC, G], BF16, tag="tp")
        for lc in range(LC):
            nc.tensor.transpose(
                at_ps[:, lc, :],
                attn_b[:, lc * 128 : (lc + 1) * 128],
                ident_bf[0:G, 0:G],
            )
        for bi in range(G):
            nc.vector.tensor_copy(atT_m[:, :, bi, bi], at_ps[:, :, bi])

        # context rows: ctx[bi, d] = sum_l attn[b0+bi, l] f[b0+bi, l, d]
        ctx_ps = ps_ctx.tile([G, D], F32, tag="ctx")
        for bi in range(G):
            for lc in range(LC):
                nc.tensor.matmul(
                    ctx_ps,
                    lhsT=atT_m[:, lc, bi, :],
                    rhs=fgrp[:, bi, lc, :],
                    start=(bi == 0 and lc == 0),
                    stop=(bi == G - 1 and lc == LC - 1),
                )
        ctx_sb = spool.tile([G, D], F32, tag="ctxsb")
        nc.vector.tensor_copy(ctx_sb, ctx_ps)
        nc.sync.dma_start(out=ctx_out.ap()[b0 : b0 + G, :], in_=ctx_sb)

    load_group(0, nchunks=4)
    for g in range(NG):
        dense_stage(g)
        if g > 0:
            reduce_stage(g - 1)
    reduce_stage(NG - 1)


_NC_CACHE = {}


def _get_nc():
    if "nc" not in _NC_CACHE:
        nc = bacc.Bacc("TRN2", target_bir_lowering=False, debug=False)
        feats = nc.declare_dram_parameter("features", [BC, L, D], F32, isOutput=False)
        hid = nc.declare_dram_parameter("hidden", [BC, H], F32, isOutput=False)
        w1 = nc.declare_dram_parameter("W1", [D, U], F32, isOutput=False)
        b1 = nc.declare_dram_parameter("b1", [U], F32, isOutput=False)
        w2 = nc.declare_dram_parameter("W2", [H, U], F32, isOutput=False)
        b2 = nc.declare_dram_parameter("b2", [U], F32, isOutput=False)
        wv = nc.declare_dram_parameter("Wv", [U, 1], F32, isOutput=False)
        ctx_out = nc.declare_dram_parameter("ctx", [BC, D], F32, isOutput=True)
        attn_out = nc.declare_dram_parameter("attn", [BC, L], F32, isOutput=True)
        from contextlib import ExitStack

        with tile.TileContext(nc) as tc, ExitStack() as es:
            _build_body(es, nc, tc, feats, hid, w1, b1, w2, b2, wv, ctx_out, attn_out)
        nc.compile()
        _NC_CACHE["nc"] = nc
    return _NC_CACHE["nc"]


def run(inputs, **spmd_kwargs):
    """Shard inputs, run on 8 NeuronCores, gather. Returns (results_obj, outputs)."""
    nc = _get_nc()
    f = np.ascontiguousarray(np.asarray(inputs["features"], dtype=np.float32))
    h = np.ascontiguousarray(np.asarray(inputs["hidden"], dtype=np.float32))
    shared = {
        "W1": np.ascontiguousarray(np.asarray(inputs["W1"], dtype=np.float32)),
        "b1": np.ascontiguousarray(np.asarray(inputs["b1"], dtype=np.float32)),
        "W2": np.ascontiguousarray(np.asarray(inputs["W2"], dtype=np.float32)),
        "b2": np.ascontiguousarray(np.asarray(inputs["b2"], dtype=np.float32)),
        "Wv": np.ascontiguousarray(np.asarray(inputs["Wv"], dtype=np.float32)),
    }
    in_maps = [
        {
            "features": f[i * BC : (i + 1) * BC],
            "hidden": h[i * BC : (i + 1) * BC],
            **shared,
        }
        for i in range(NCORES)
    ]
    res = run_bass_kernel_spmd(nc, in_maps, core_ids=list(range(NCORES)), **spmd_kwargs)
    ctx = np.concatenate([r["ctx"] for r in res.results], axis=0).astype(np.float32)
    attn = np.concatenate([r["attn"] for r in res.results], axis=0)[..., None].astype(
        np.float32
    )
    return res, (ctx, attn)


def kernel(features, hidden, W1, b1, W2, b2, Wv, bv):
    del bv  # softmax(logits + bv) == softmax(logits); bv never affects outputs
    _, out = run(
        {
            "features": features,
            "hidden": hidden,
            "W1": W1,
            "b1": b1,
            "W2": W2,
            "b2": b2,
            "Wv": Wv,
        }
    )
    return out


# revision 36
# speedup vs baseline: 1.0587x; 1.0004x over previous
"""Trainium2 Bass kernel for additive-attention (nn_Attention_5789615915550).

reference math (per batch b):
    proj_f = features @ W1 + b1            # [L, U]
    proj_h = hidden[b] @ W2 + b2           # [U]
    score  = tanh(proj_f + proj_h)         # [L, U]
    logits = score @ Wv (+ bv)             # [L, 1]
    attn   = softmax(logits, axis=L)       # [L, 1]   (bv cancels in softmax)
    ctx    = sum_l attn[l] * features[l]   # [D]

Sharding: pure data parallel, batch dim B=1024 split across 8 cores (128 each).
All params replicated. Outputs gathered/reassembled on host.

Per-core dataflow (all matmul operands bf16, fp32 accumulation in PSUM):
  - features are cast-DMA'd (fp32->bf16) in groups of G=16 batches,
    natural layout [l_part, b, l_chunk, d].
  - PE transpose gives fT [d_part, d_chunk, l] per batch (rhs of W1 matmul).
  - proj psum [u_sub, u_chunk, l]; tanh on ACT with per-partition bias
    projh[u, b] (+b1+b2) fused via the activation bias operand.
  - logits: lhsT is a masked replica of Wv (only column bi nonzero), so all
    G batches accumulate their logit row into one [G, L] psum tile -- no
    partition-shifting copies needed.
  - softmax over L for G rows at once (exp's accum_out gives the sum free).
  - context: per batch 4 small matmuls, f-block as stationary weights,
    transposed attention column as moving operand; result collected as
    ctxT [D, B_c] and transposed on host.
"""

import os
import sys

import numpy as np

for _p in ("/opt/trn_rl_repo", "/root/.axon_site/_ro/trn_rl_repo"):
    if os.path.isdir(_p) and _p not in sys.path:
        sys.path.append(_p)

import concourse.bacc as bacc
import concourse.bass as bass
import concourse.tile as tile
from concourse import mybir
from concourse.bass_utils import run_bass_kernel_spmd
from concourse.masks import make_identity

B, L, D, H, U = 1024, 256, 256, 512, 512
NCORES = 8
BC = B // NCORES          # batches per core
G = 16                    # batches per softmax group
NG = BC // G              # groups per core
DC, UC, HC, LC = D // 128, U // 128, H // 128, L // 128  # 2, 4, 4, 2

F32 = mybir.dt.float32
BF16 = mybir.dt.bfloat16
AF = mybir.ActivationFunctionType


def _build_body(ctx, nc, tc, feats, hid, w1, b1, w2, b2, wv, ctx_out, attn_out):
    consts = ctx.enter_context(tc.tile_pool(name="consts", bufs=1))
    fpool = ctx.enter_context(tc.tile_pool(name="fpool", bufs=4))
    work = ctx.enter_context(tc.tile_pool(name="work", bufs=4))
    spool = ctx.enter_context(tc.tile_pool(name="spool", bufs=3))
    ps_tp = ctx.enter_context(tc.tile_pool(name="ps_tp", bufs=2, space="PSUM"))
    # proj psum in 1-bank tiles (2 u-chunks each) -> finer-grained release,
    # so tanh of one half unblocks the next batch's matmuls earlier.
    ps_proj = ctx.enter_context(tc.tile_pool(name="ps_proj", bufs=2, space="PSUM"))
    # logits accumulator and context rows have back-to-back lifetimes; share
    # one pool/tag so both fit in 2 banks.
    ps_log = ctx.enter_context(tc.tile_pool(name="ps_log", bufs=1, space="PSUM"))
    ps_ctx = ctx.enter_context(tc.tile_pool(name="ps_ctx", bufs=1, space="PSUM"))

    # ---- constants / setup -------------------------------------------------
    ident_bf = consts.tile([128, 128], BF16)
    make_identity(nc, ident_bf)

    w1_sb = consts.tile([128, DC, U], BF16)  # [d_sub, d_chunk, u]
    nc.gpsimd.dma_start(out=w1_sb, in_=w1.ap().rearrange("(c p) u -> p c u", p=128))
    w2_sb = consts.tile([128, HC, U], BF16)  # [h_sub, h_chunk, u]
    nc.gpsimd.dma_start(out=w2_sb, in_=w2.ap().rearrange("(c p) u -> p c u", p=128))

    wv_f32 = consts.tile([128, UC, 1], F32)  # [u_sub, u_chunk, 1]
    nc.sync.dma_start(out=wv_f32, in_=wv.ap().rearrange("(c p) o -> p c o", p=128))
    b1_sb = consts.tile([128, UC], F32)
    nc.sync.dma_start(out=b1_sb, in_=b1.ap().rearrange("(c p) -> p c", p=128))
    b2_sb = consts.tile([128, UC], F32)
    nc.sync.dma_start(out=b2_sb, in_=b2.ap().rearrange("(c p) -> p c", p=128))
    bias12 = consts.tile([128, UC], F32)
    nc.vector.tensor_add(bias12, b1_sb, b2_sb)

    # Wv masked replicas: wv_mask[p, bi, c, j] = Wv[c*128+p] * (j == bi)
    wv_mask = consts.tile([128, G, UC, G], BF16)
    nc.vector.memset(wv_mask, 0.0)
    for bi in range(G):
        nc.vector.tensor_copy(wv_mask[:, bi, :, bi : bi + 1], wv_f32)

    # hidden -> hT (bf16), then projh[u, b] = W2.T @ hidden.T  (+ b1 + b2)
    hid_sb = consts.tile([128, H], BF16)  # [b, h]
    nc.gpsimd.dma_start(out=hid_sb, in_=hid.ap())
    hT_ps = ps_tp.tile([128, HC, 128], BF16, tag="tp")
    for hc in range(HC):
        nc.tensor.transpose(
            hT_ps[:, hc, :], hid_sb[:, hc * 128 : (hc + 1) * 128], ident_bf
        )
    hT_sb = consts.tile([128, HC, 128], BF16)  # [h_sub, h_chunk, b]
    nc.vector.tensor_copy(hT_sb, hT_ps)

    projh_sb = consts.tile([128, UC, BC], F32)  # [u_sub, u_chunk, b]
    projh_ps = ps_proj.tile([128, UC, 128], F32, tag="proj")
    for uc in range(UC):
        for hc in range(HC):
            nc.tensor.matmul(
                projh_ps[:, uc, :],
                lhsT=w2_sb[:, hc, uc * 128 : (uc + 1) * 128],
                rhs=hT_sb[:, hc, :],
                start=(hc == 0),
                stop=(hc == HC - 1),
            )
    for uc in range(UC):
        nc.vector.tensor_scalar_add(
            projh_sb[:, uc, :], projh_ps[:, uc, :], bias12[:, uc : uc + 1]
        )

    # Masked transposed-attention weights: atT_m[p, lc, bi, j] is nonzero only
    # at j == bi, holding attnT for batch b0+bi. Zeroed once; only the diagonal
    # columns are ever rewritten, so the zeros persist across groups.
    atT_m = consts.tile([128, LC, G, G], BF16)
    nc.vector.memset(atT_m, 0.0)

    # ---- main loop (software-pipelined: group g dense work, then group g-1
    # softmax/context which overlaps with it) ------------------------------
    fgrps = {}
    logaccs = {}

    def load_group(g, nchunks=2):
        fgrp = fpool.tile([128, G, LC, D], BF16, tag="fgrp")  # [l_sub, b, lc, d]
        fgrps[g] = fgrp
        b0 = g * G
        step = G // nchunks
        for h in range(nchunks):
            nc.gpsimd.dma_start(
                out=fgrp[:, h * step : (h + 1) * step],
                in_=feats.ap()[b0 + h * step : b0 + (h + 1) * step].rearrange(
                    "b (c p) d -> p b c d", p=128
                ),
            )

    def dense_stage(g):
        b0 = g * G
        if g + 1 < NG:
            load_group(g + 1)
        fgrp = fgrps[g]
        logacc_ps = ps_log.tile([G, L], F32, tag="log")
        logaccs[g] = logacc_ps

        def emit_logits(bi, score):
            for uc in range(UC):
                nc.tensor.matmul(
                    logacc_ps,
                    lhsT=wv_mask[:, bi, uc, :],
                    rhs=score[:, uc, :],
                    start=(bi == 0 and uc == 0),
                    stop=(bi == G - 1 and uc == UC - 1),
                )

        pending = None  # (bi, score) whose logits are deferred one batch
        for bi in range(G):
            b = b0 + bi
            # transpose f[b]: [l, d] -> fT [d_sub, d_chunk, l]
            ft_ps = ps_tp.tile([128, DC, L], BF16, tag="tp")
            for lc in range(LC):
                for dc in range(DC):
                    nc.tensor.transpose(
                        ft_ps[:, dc, lc * 128 : (lc + 1) * 128],
                        fgrp[:, bi, lc, dc * 128 : (dc + 1) * 128],
                        ident_bf,
                    )
            ft_sb = work.tile([128, DC, L], BF16, tag="ft")
            nc.vector.tensor_copy(ft_sb, ft_ps)

            # proj_f[u, l] for this batch
            proj_ps = ps_proj.tile([128, UC, L], F32, tag="proj")
            for uc in range(UC):
                for dc in range(DC):
                    nc.tensor.matmul(
                        proj_ps[:, uc, :],
                        lhsT=w1_sb[:, dc, uc * 128 : (uc + 1) * 128],
                        rhs=ft_sb[:, dc, :],
                        start=(dc == 0),
                        stop=(dc == DC - 1),
                    )
            # score = tanh(proj_f + projh[:, b] + b1 + b2)
            score = work.tile([128, UC, L], BF16, tag="score")
            for uc in range(UC):
                nc.scalar.activation(
                    out=score[:, uc, :],
                    in_=proj_ps[:, uc, :],
                    func=AF.Tanh,
                    bias=projh_sb[:, uc, b : b + 1],
                    scale=1.0,
                )

            # logits lag one batch so tanh(b) overlaps proj(b+1) on the PE
            if pending is not None:
                emit_logits(*pending)
            pending = (bi, score)
        emit_logits(*pending)

    def reduce_stage(g):
        b0 = g * G
        logacc_ps = logaccs.pop(g)
        fgrp = fgrps.pop(g)
        # ---- batched softmax over L for the G rows (read psum directly) ----
        mx = spool.tile([G, 1], F32, tag="mx")
        nc.vector.reduce_max(mx, logacc_ps, axis=mybir.AxisListType.X)
        nmx = spool.tile([G, 1], F32, tag="nmx")
        nc.vector.tensor_scalar_mul(nmx, mx, -1.0)
        probs = spool.tile([G, L], F32, tag="probs")
        ssum = spool.tile([G, 1], F32, tag="ssum")
        nc.scalar.activation(
            out=probs,
            in_=logacc_ps,
            func=AF.Exp,
            bias=nmx,
            scale=1.0,
            accum_out=ssum,
        )
        rec = spool.tile([G, 1], F32, tag="rec")
        nc.vector.reciprocal(rec, ssum)
        attn_f = spool.tile([G, L], F32, tag="attnf")
        nc.vector.tensor_scalar_mul(attn_f, probs, rec)
        nc.sync.dma_start(out=attn_out.ap()[b0 : b0 + G, :], in_=attn_f)
        attn_b = spool.tile([G, L], BF16, tag="attnb")
        nc.vector.tensor_scalar_mul(attn_b, probs, rec)

        # attn -> attnT [l_sub, l_chunk, b_in_group], scattered to diagonal
        at_ps = ps_tp.tile([128, LC, G], BF16, tag="tp")
        for lc in range(LC):
            nc.tensor.transpose(
                at_ps[:, lc, :],
                attn_b[:, lc * 128 : (lc + 1) * 128],
                ident_bf[0:G, 0:G],
            )
        for bi in range(G):
            nc.vector.tensor_copy(atT_m[:, :, bi, bi], at_ps[:, :, bi])

        # context rows: ctx[bi, d] = sum_l attn[b0+bi, l] f[b0+bi, l, d]
        ctx_ps = ps_ctx.tile([G, D], F32, tag="ctx")
        for bi in range(G):
            for lc in range(LC):
                nc.tensor.matmul(
                    ctx_ps,
                    lhsT=atT_m[:, lc, bi, :],
                    rhs=fgrp[:, bi, lc, :],
                    start=(bi == 0 and lc == 0),
                    stop=(bi == G - 1 and lc == LC - 1),
                )
        ctx_sb = spool.tile([G, D], F32, tag="ctxsb")
        nc.vector.tensor_copy(ctx_sb, ctx_ps)
        nc.sync.dma_start(out=ctx_out.ap()[b0 : b0 + G, :], in_=ctx_sb)

    load_group(0, nchunks=4)
    for g in range(NG):
        dense_stage(g)
        if g > 0:
            reduce_stage(g - 1)
    reduce_stage(NG - 1)


_NC_CACHE = {}


def _get_nc():
    if "nc" not in _NC_CACHE:
        nc = bacc.Bacc("TRN2", target_bir_lowering=False, debug=False)
        feats = nc.declare_dram_parameter("features", [BC, L, D], F32, isOutput=False)
        hid = nc.declare_dram_parameter("hidden", [BC, H], F32, isOutput=False)
        w1 = nc.declare_dram_parameter("W1", [D, U], F32, isOutput=False)
        b1 = nc.declare_dram_parameter("b1", [U], F32, isOutput=False)
        w2 = nc.declare_dram_parameter("W2", [H, U], F32, isOutput=False)
        b2 = nc.declare_dram_parameter("b2", [U], F32, isOutput=False)
        wv = nc.declare_dram_parameter("Wv", [U, 1], F32, isOutput=False)
        ctx_out = nc.declare_dram_parameter("ctx", [BC, D], F32, isOutput=True)
        attn_out = nc.declare_dram_parameter("attn", [BC, L], F32, isOutput=True)
        from contextlib import ExitStack

        with tile.TileContext(nc) as tc, ExitStack() as es:
            _build_body(es, nc, tc, feats, hid, w1, b1, w2, b2, wv, ctx_out, attn_out)
        nc.compile()
        _NC_CACHE["nc"] = nc
    return _NC_CACHE["nc"]


def run(inputs, **spmd_kwargs):
    """Shard inputs, run on 8 NeuronCores, gather. Returns (results_obj, outputs)."""
    nc = _get_nc()
    f = np.ascontiguousarray(np.asarray(inputs["features"], dtype=np.float32))
    h = np.ascontiguousarray(np.asarray(inputs["hidden"], dtype=np.float32))
    shared = {
        "W1": np.ascontiguousarray(np.asarray(inputs["W1"], dtype=np.float32)),
        "b1": np.ascontiguousarray(np.asarray(inputs["b1"], dtype=np.float32)),
        "W2": np.ascontiguousarray(np.asarray(inputs["W2"], dtype=np.float32)),
        "b2": np.ascontiguousarray(np.asarray(inputs["b2"], dtype=np.float32)),
        "Wv": np.ascontiguousarray(np.asarray(inputs["Wv"], dtype=np.float32)),
    }
    in_maps = [
        {
            "features": f[i * BC : (i + 1) * BC],
            "hidden": h[i * BC : (i + 1) * BC],
            **shared,
        }
        for i in range(NCORES)
    ]
    res = run_bass_kernel_spmd(nc, in_maps, core_ids=list(range(NCORES)), **spmd_kwargs)
    ctx = np.concatenate([r["ctx"] for r in res.results], axis=0).astype(np.float32)
    attn = np.concatenate([r["attn"] for r in res.results], axis=0)[..., None].astype(
        np.float32
    )
    return res, (ctx, attn)


def kernel(features, hidden, W1, b1, W2, b2, Wv, bv):
    del bv  # softmax(logits + bv) == softmax(logits); bv never affects outputs
    _, out = run(
        {
            "features": features,
            "hidden": hidden,
            "W1": W1,
            "b1": b1,
            "W2": W2,
            "b2": b2,
            "Wv": Wv,
        }
    )
    return out
